# revision 7
# baseline (speedup 1.0000x reference)
"""CodaPrompt kernel for Trainium2 (Bass/Tile), data-parallel over batch on 8 NeuronCores.

Math (reference):
    a[e,b,k,:] = x[b,:] * As[e,k,:]
    q = a / max(||a||_2, eps)        (normalize over d)
    nK = Ks / max(||Ks||_2, eps)
    aq[e,b,k] = <q[e,b,k,:], nK[e,k,:]>
    P_[e,b,l,:] = sum_k aq[e,b,k] * Ps[e,k,l,:]
    out = stack([P_[:,:, :L/2], P_[:,:, L/2:]])   # [2, E, B, L/2, D]

Device-side formulation (per batch shard of BC rows):
    num[e,k,b] = sum_d (As*nK)[e,k,d] * x[b,d]        -> matmul, contraction over d
    den2[e,k,b] = sum_d (As*As)[e,k,d] * x2[b,d]      -> matmul
    aq[e,k,b] = num * rsqrt(den2)                      (ACT sqrt + DVE reciprocal + mul)
    P_[b, (l d)] = aq[e,:,b].T @ Ps[e]                 -> matmul, contraction over k

Host prep is limited to O(E*K*D) pool preprocessing (normalize Ks, squares,
transposes) and the batch-shard transpose of x; all O(B*...) FLOPs run on device.
"""

import os
import sys
from contextlib import ExitStack

import numpy as np

if "/opt/trn_rl_repo" not in sys.path:
    sys.path.insert(0, "/opt/trn_rl_repo")

import concourse.bass as bass
import concourse.mybir as mybir
from concourse import bacc, tile
from concourse.bass_utils import run_bass_kernel_spmd

B, D, E, K, L = 2048, 768, 5, 100, 8
NCORES = 8
BC = B // NCORES          # 256 batch rows per core
DC = D // 128             # 6 contraction chunks of 128
ND = L * D                # 6144
NCHUNK = 512              # psum bank width in f32
NJ = ND // NCHUNK         # 12 n-chunks
MC = BC // 128            # 2 output-partition chunks
EPS = 1e-12

F32 = mybir.dt.float32
# "float32r" = single-pass reduced-precision fp32 matmul (full PE rate at N>=256);
# "float32" = exact but 4 cycles/row. Flip here after measuring accuracy.
MM_DTYPE = os.environ.get("CODA_MM_DTYPE", "float32")
MM_DT = getattr(mybir.dt, MM_DTYPE)


def _build_bass(repeat=1):
    # Bacc (not plain Bass): its finalize() runs move_matmul_waits_to_ldweights
    # + generate_event_semaphores, without which multi-dependency matmuls hit
    # walrus "Too many sync wait commands".
    # `repeat` replicates the whole compute body (timing instrumentation:
    # slope over repeat removes per-launch overhead); results are idempotent.
    nc = bacc.Bacc(None)

    # Matmul operands must be produced as MM_DT end-to-end (walrus verifies
    # fp32r consumers see fp32r producers). float32r is bit-identical to
    # float32 in DRAM, so host arrays stay np.float32 either way.
    xT_d = nc.declare_dram_parameter("xT", [D, BC], MM_DT, isOutput=False)
    x2T_d = nc.declare_dram_parameter("x2T", [D, BC], MM_DT, isOutput=False)
    w_d = nc.declare_dram_parameter("w12T", [D, 2, E, K], MM_DT, isOutput=False)
    ps_d = nc.declare_dram_parameter("ps", [E, K, ND], MM_DT, isOutput=False)
    out_d = nc.declare_dram_parameter("out", [2, E, BC, L // 2, D], F32, isOutput=True)

    with ExitStack() as ctx:
        tc = ctx.enter_context(tile.TileContext(nc))
        const = ctx.enter_context(tc.tile_pool(name="const", bufs=1))
        psp = ctx.enter_context(tc.tile_pool(name="psp", bufs=2))
        smallp = ctx.enter_context(tc.tile_pool(name="smallp", bufs=2))
        resp = ctx.enter_context(tc.tile_pool(name="resp", bufs=4))
        pndp = ctx.enter_context(tc.tile_pool(name="pndp", bufs=2, space="PSUM"))
        ppp = ctx.enter_context(tc.tile_pool(name="ppp", bufs=4, space="PSUM"))

        # Resident operands: x shard (transposed), its square, and the fused
        # W1=As*nK / W2=As^2 weight block, all chunked to 128 partitions.
        xs = const.tile([128, DC, BC], MM_DT, name="xs", tag="xs")
        nc.sync.dma_start(xs[:], xT_d[:].rearrange("(c p) b -> p c b", p=128))
        x2s = const.tile([128, DC, BC], MM_DT, name="x2s", tag="x2s")
        nc.sync.dma_start(x2s[:], x2T_d[:].rearrange("(c p) b -> p c b", p=128))
        ws = const.tile([128, DC, 2, E, K], MM_DT, name="ws", tag="ws")
        nc.sync.dma_start(ws[:], w_d[:].rearrange("(c p) t e k -> p c t e k", p=128))

        for e in [e for _ in range(repeat) for e in range(E)]:
            pst = psp.tile([K, ND], MM_DT, name="pst", tag="ps")
            nc.sync.dma_start(pst[:], ps_d[e])

            num = pndp.tile([K, BC], F32, name="num", tag="num")
            den = pndp.tile([K, BC], F32, name="den", tag="den")
            for c in range(DC):
                nc.tensor.matmul(
                    num[:],
                    ws[:, c, 0, e, :],
                    xs[:, c, :],
                    start=(c == 0),
                    stop=(c == DC - 1),
                )
            for c in range(DC):
                nc.tensor.matmul(
                    den[:],
                    ws[:, c, 1, e, :],
                    x2s[:, c, :],
                    start=(c == 0),
                    stop=(c == DC - 1),
                )

            # aq = num / sqrt(den2)   (den2 >> eps^2 for this regime)
            sden = smallp.tile([K, BC], F32, name="sden", tag="sden")
            nc.scalar.sqrt(sden[:], den[:])
            rden = smallp.tile([K, BC], F32, name="rden", tag="rden")
            nc.vector.reciprocal(rden[:], sden[:])
            aq = smallp.tile([K, BC], MM_DT, name="aq", tag="aq")
            nc.vector.tensor_mul(aq[:], num[:], rden[:])

            for m in range(MC):
                for s in range(2):
                    res = resp.tile([128, ND // 2], F32, name="res", tag="res")
                    for j in range(NJ // 2):
                        pp = ppp.tile([128, NCHUNK], F32, name="pp", tag="pp")
                        col = (s * (NJ // 2) + j) * NCHUNK
                        nc.tensor.matmul(
                            pp[:],
                            aq[:, m * 128 : (m + 1) * 128],
                            pst[:, col : col + NCHUNK],
                            start=True,
                            stop=True,
                        )
                        dst = res[:, j * NCHUNK : (j + 1) * NCHUNK]
                        if j % 2 == 0:
                            nc.vector.tensor_copy(dst, pp[:])
                        else:
                            nc.scalar.copy(dst, pp[:])
                    out_ap = out_d[s, e, m * 128 : (m + 1) * 128, :, :]
                    nc.sync.dma_start(out_ap.rearrange("b l d -> b (l d)"), res[:])

    if not nc.is_finalized():
        nc.finalize()
    return nc


_NC_CACHE = None


def _get_nc():
    global _NC_CACHE
    if _NC_CACHE is None:
        _NC_CACHE = _build_bass()
    return _NC_CACHE


def _prep_inputs(x, Ks, As, Ps):
    x = np.asarray(x, dtype=np.float32)
    Ks = np.asarray(Ks, dtype=np.float32)
    As = np.asarray(As, dtype=np.float32)
    Ps = np.asarray(Ps, dtype=np.float32)

    nrm = np.sqrt(np.sum(Ks * Ks, axis=-1, keepdims=True))
    nK = Ks / np.maximum(nrm, EPS)
    w12T = np.empty((D, 2, E, K), dtype=np.float32)
    w12T[:, 0] = (As * nK).transpose(2, 0, 1)
    w12T[:, 1] = (As * As).transpose(2, 0, 1)

    ps2 = np.ascontiguousarray(Ps.reshape(E, K, ND))
    xT = np.ascontiguousarray(x.T)          # [D, B]
    x2T = xT * xT

    in_maps = []
    for c in range(NCORES):
        sl = slice(c * BC, (c + 1) * BC)
        in_maps.append(
            {
                "xT": np.ascontiguousarray(xT[:, sl]),
                "x2T": np.ascontiguousarray(x2T[:, sl]),
                "w12T": w12T,
                "ps": ps2,
            }
        )
    return in_maps


def _run(x, Ks, As, Ps, trace=False, **spmd_kwargs):
    nc = _get_nc()
    in_maps = _prep_inputs(x, Ks, As, Ps)
    res = run_bass_kernel_spmd(nc, in_maps, list(range(NCORES)), trace=trace, **spmd_kwargs)
    out = np.empty((2, E, B, L // 2, D), dtype=np.float32)
    for c in range(NCORES):
        out[:, :, c * BC : (c + 1) * BC] = res.results[c]["out"]
    return out, res


def kernel(x, Ks, As, Ps):
    out, _ = _run(x, Ks, As, Ps, trace=False)
    return out


# revision 8
# speedup vs baseline: 122.7188x; 122.7188x over previous
"""CodaPrompt kernel for Trainium2 (Bass/Tile), data-parallel over batch on 8 NeuronCores.

Math (reference):
    a[e,b,k,:] = x[b,:] * As[e,k,:]
    q = a / max(||a||_2, eps)        (normalize over d)
    nK = Ks / max(||Ks||_2, eps)
    aq[e,b,k] = <q[e,b,k,:], nK[e,k,:]>
    P_[e,b,l,:] = sum_k aq[e,b,k] * Ps[e,k,l,:]
    out = stack([P_[:,:, :L/2], P_[:,:, L/2:]])   # [2, E, B, L/2, D]

Device-side formulation (per batch shard of BC rows):
    num[e,k,b] = sum_d (As*nK)[e,k,d] * x[b,d]        -> matmul, contraction over d
    den2[e,k,b] = sum_d (As*As)[e,k,d] * x2[b,d]      -> matmul
    aq[e,k,b] = num * rsqrt(den2)                      (ACT sqrt + DVE reciprocal + mul)
    P_[b, (l d)] = aq[e,:,b].T @ Ps[e]                 -> matmul, contraction over k

Host prep is limited to O(E*K*D) pool preprocessing (normalize Ks, squares,
transposes) and the batch-shard transpose of x; all O(B*...) FLOPs run on device.
"""

import os
import sys
from contextlib import ExitStack

import numpy as np

if "/opt/trn_rl_repo" not in sys.path:
    sys.path.insert(0, "/opt/trn_rl_repo")

import concourse.bass as bass
import concourse.mybir as mybir
from concourse import bacc, tile
from concourse.bass_utils import run_bass_kernel_spmd

B, D, E, K, L = 2048, 768, 5, 100, 8
NCORES = 8
BC = B // NCORES          # 256 batch rows per core
DC = D // 128             # 6 contraction chunks of 128
ND = L * D                # 6144
NCHUNK = 512              # psum bank width in f32
NJ = ND // NCHUNK         # 12 n-chunks
MC = BC // 128            # 2 output-partition chunks
EPS = 1e-12

F32 = mybir.dt.float32
# "float32r" = single-pass reduced-precision fp32 matmul (full PE rate at N>=256);
# "float32" = exact but 4 cycles/row. Flip here after measuring accuracy.
MM_DTYPE = os.environ.get("CODA_MM_DTYPE", "float32r")
MM_DT = getattr(mybir.dt, MM_DTYPE)


def _build_bass(repeat=1):
    # Bacc (not plain Bass): its finalize() runs move_matmul_waits_to_ldweights
    # + generate_event_semaphores, without which multi-dependency matmuls hit
    # walrus "Too many sync wait commands".
    # `repeat` replicates the whole compute body (timing instrumentation:
    # slope over repeat removes per-launch overhead); results are idempotent.
    nc = bacc.Bacc(None)

    # Matmul operands must be produced as MM_DT end-to-end (walrus verifies
    # fp32r consumers see fp32r producers). float32r is bit-identical to
    # float32 in DRAM, so host arrays stay np.float32 either way.
    xT_d = nc.declare_dram_parameter("xT", [D, BC], MM_DT, isOutput=False)
    x2T_d = nc.declare_dram_parameter("x2T", [D, BC], MM_DT, isOutput=False)
    w_d = nc.declare_dram_parameter("w12T", [D, 2, E, K], MM_DT, isOutput=False)
    ps_d = nc.declare_dram_parameter("ps", [E, K, ND], MM_DT, isOutput=False)
    out_d = nc.declare_dram_parameter("out", [2, E, BC, L // 2, D], F32, isOutput=True)

    with ExitStack() as ctx:
        tc = ctx.enter_context(tile.TileContext(nc))
        const = ctx.enter_context(tc.tile_pool(name="const", bufs=1))
        psp = ctx.enter_context(tc.tile_pool(name="psp", bufs=2))
        smallp = ctx.enter_context(tc.tile_pool(name="smallp", bufs=2))
        resp = ctx.enter_context(tc.tile_pool(name="resp", bufs=4))
        pndp = ctx.enter_context(tc.tile_pool(name="pndp", bufs=2, space="PSUM"))
        ppp = ctx.enter_context(tc.tile_pool(name="ppp", bufs=4, space="PSUM"))

        # Resident operands: x shard (transposed), its square, and the fused
        # W1=As*nK / W2=As^2 weight block, all chunked to 128 partitions.
        xs = const.tile([128, DC, BC], MM_DT, name="xs", tag="xs")
        nc.sync.dma_start(xs[:], xT_d[:].rearrange("(c p) b -> p c b", p=128))
        x2s = const.tile([128, DC, BC], MM_DT, name="x2s", tag="x2s")
        nc.sync.dma_start(x2s[:], x2T_d[:].rearrange("(c p) b -> p c b", p=128))
        ws = const.tile([128, DC, 2, E, K], MM_DT, name="ws", tag="ws")
        nc.sync.dma_start(ws[:], w_d[:].rearrange("(c p) t e k -> p c t e k", p=128))

        for e in [e for _ in range(repeat) for e in range(E)]:
            pst = psp.tile([K, ND], MM_DT, name="pst", tag="ps")
            nc.sync.dma_start(pst[:], ps_d[e])

            num = pndp.tile([K, BC], F32, name="num", tag="num")
            den = pndp.tile([K, BC], F32, name="den", tag="den")
            for c in range(DC):
                nc.tensor.matmul(
                    num[:],
                    ws[:, c, 0, e, :],
                    xs[:, c, :],
                    start=(c == 0),
                    stop=(c == DC - 1),
                )
            for c in range(DC):
                nc.tensor.matmul(
                    den[:],
                    ws[:, c, 1, e, :],
                    x2s[:, c, :],
                    start=(c == 0),
                    stop=(c == DC - 1),
                )

            # aq = num / sqrt(den2)   (den2 >> eps^2 for this regime)
            sden = smallp.tile([K, BC], F32, name="sden", tag="sden")
            nc.scalar.sqrt(sden[:], den[:])
            rden = smallp.tile([K, BC], F32, name="rden", tag="rden")
            nc.vector.reciprocal(rden[:], sden[:])
            aq = smallp.tile([K, BC], MM_DT, name="aq", tag="aq")
            nc.vector.tensor_mul(aq[:], num[:], rden[:])

            for m in range(MC):
                for s in range(2):
                    res = resp.tile([128, ND // 2], F32, name="res", tag="res")
                    for j in range(NJ // 2):
                        pp = ppp.tile([128, NCHUNK], F32, name="pp", tag="pp")
                        col = (s * (NJ // 2) + j) * NCHUNK
                        nc.tensor.matmul(
                            pp[:],
                            aq[:, m * 128 : (m + 1) * 128],
                            pst[:, col : col + NCHUNK],
                            start=True,
                            stop=True,
                        )
                        dst = res[:, j * NCHUNK : (j + 1) * NCHUNK]
                        if j % 2 == 0:
                            nc.vector.tensor_copy(dst, pp[:])
                        else:
                            nc.scalar.copy(dst, pp[:])
                    out_ap = out_d[s, e, m * 128 : (m + 1) * 128, :, :]
                    nc.sync.dma_start(out_ap.rearrange("b l d -> b (l d)"), res[:])

    if not nc.is_finalized():
        nc.finalize()
    return nc


_NC_CACHE = None


def _get_nc():
    global _NC_CACHE
    if _NC_CACHE is None:
        _NC_CACHE = _build_bass()
    return _NC_CACHE


def _prep_inputs(x, Ks, As, Ps):
    x = np.asarray(x, dtype=np.float32)
    Ks = np.asarray(Ks, dtype=np.float32)
    As = np.asarray(As, dtype=np.float32)
    Ps = np.asarray(Ps, dtype=np.float32)

    nrm = np.sqrt(np.sum(Ks * Ks, axis=-1, keepdims=True))
    nK = Ks / np.maximum(nrm, EPS)
    w12T = np.empty((D, 2, E, K), dtype=np.float32)
    w12T[:, 0] = (As * nK).transpose(2, 0, 1)
    w12T[:, 1] = (As * As).transpose(2, 0, 1)

    ps2 = np.ascontiguousarray(Ps.reshape(E, K, ND))
    xT = np.ascontiguousarray(x.T)          # [D, B]
    x2T = xT * xT

    in_maps = []
    for c in range(NCORES):
        sl = slice(c * BC, (c + 1) * BC)
        in_maps.append(
            {
                "xT": np.ascontiguousarray(xT[:, sl]),
                "x2T": np.ascontiguousarray(x2T[:, sl]),
                "w12T": w12T,
                "ps": ps2,
            }
        )
    return in_maps


def _run(x, Ks, As, Ps, trace=False, **spmd_kwargs):
    nc = _get_nc()
    in_maps = _prep_inputs(x, Ks, As, Ps)
    res = run_bass_kernel_spmd(nc, in_maps, list(range(NCORES)), trace=trace, **spmd_kwargs)
    out = np.empty((2, E, B, L // 2, D), dtype=np.float32)
    for c in range(NCORES):
        out[:, :, c * BC : (c + 1) * BC] = res.results[c]["out"]
    return out, res


def kernel(x, Ks, As, Ps):
    out, _ = _run(x, Ks, As, Ps, trace=False)
    return out
